# revision 1
# baseline (speedup 1.0000x reference)
"""Trainium2 Bass kernel for LFGA-style attention block (raw Bass, 8-core SPMD).

Per-batch (B=8, C=256, H=W=64, N=4096, CQ=64), one batch element per core:
    q/k = Wq/Wk @ fb + b   [64, N];  v = Wv @ fa + bv  [C, N]
    S2[j,i] = k.q (energy TRANSPOSED so softmax dim j is on partitions)
    A2 = exp(S2 + bias);  O_un[c,i] = sum_j vT[j,c] A2[j,i]
    s[i] = sum_j A2[j,i] (DVE chunk-accumulate + ones-matmul partition reduce)
    out = relu(gamma/s * O_un + fa)
"""

import numpy as np

import concourse.bass as bass
import concourse.mybir as mybir
from concourse.bass_utils import run_bass_kernel_spmd

P = 128
B, C, HW = 8, 256, 64
N = HW * HW
CQ = 64
NT = 512
NIT = N // NT        # 8
NJ = N // P          # 32
F32 = mybir.dt.float32
EXP_BIAS = -20.0
AF = mybir.ActivationFunctionType

# engine stream bases / sizes
DS0 = 9 * 16                 # dsem after input loads
TQKV = 32 + 96               # PE matmuls in qkv phase
PEIT = 98                    # PE matmuls per i-tile
AQKV = 16 + 32               # ACT ops in qkv phase
AIT = 35                     # ACT ops per i-tile
VS0 = 3                      # DVE memsets
VIT = 38                     # DVE ops per i-tile

_CACHE = {}


def _pos_s2(jj):
    return jj + 1 if jj < 2 else 3 * jj - 3


def _pos_oc1(jb):
    return 3 * jb + 5 if jb <= 29 else (94 if jb == 30 else 96)


def _build():
    nc = bass.Bass()

    fa = nc.declare_dram_parameter("fa", [C, N], F32, isOutput=False)
    fb = nc.declare_dram_parameter("fb", [C, N], F32, isOutput=False)
    wqT = nc.declare_dram_parameter("wqT", [C, CQ], F32, isOutput=False)
    wkT = nc.declare_dram_parameter("wkT", [C, CQ], F32, isOutput=False)
    wvT = nc.declare_dram_parameter("wvT", [C, C], F32, isOutput=False)
    bqd = nc.declare_dram_parameter("bq", [CQ, 1], F32, isOutput=False)
    bkd = nc.declare_dram_parameter("bk", [CQ, 1], F32, isOutput=False)
    bvd = nc.declare_dram_parameter("bv", [1, C], F32, isOutput=False)
    gamd = nc.declare_dram_parameter("gamma", [P, 1], F32, isOutput=False)
    out = nc.declare_dram_parameter("out", [C, N], F32, isOutput=True)

    fa3 = fa.rearrange("(o p) n -> p o n", p=P)
    fb3 = fb.rearrange("(o p) n -> p o n", p=P)
    wq3 = wqT.rearrange("(o p) m -> p o m", p=P)
    wk3 = wkT.rearrange("(o p) m -> p o m", p=P)
    wv3 = wvT.rearrange("(o p) m -> p o m", p=P)
    out3 = out.rearrange("(o p) n -> p o n", p=P)

    def T0(it):
        return TQKV + PEIT * it

    def A0(it):
        return AQKV + AIT * it

    def V0(it):
        return VS0 + VIT * it

    from contextlib import ExitStack
    with ExitStack() as _es:
        fa_sb = _es.enter_context(nc.sbuf_tensor([P, 2, N], F32))
        fb_sb = _es.enter_context(nc.sbuf_tensor([P, 2, N], F32))
        wq_sb = _es.enter_context(nc.sbuf_tensor([P, 2, CQ], F32))
        wk_sb = _es.enter_context(nc.sbuf_tensor([P, 2, CQ], F32))
        wv_sb = _es.enter_context(nc.sbuf_tensor([P, 2, C], F32))
        bq_sb = _es.enter_context(nc.sbuf_tensor([CQ, 1], F32))
        bk_sb = _es.enter_context(nc.sbuf_tensor([CQ, 1], F32))
        bv_sb = _es.enter_context(nc.sbuf_tensor([1, C], F32))
        gam_sb = _es.enter_context(nc.sbuf_tensor([P, 1], F32))
        onesc = _es.enter_context(nc.sbuf_tensor([P, 1], F32))
        onesr = _es.enter_context(nc.sbuf_tensor([1, P], F32))
        expb = _es.enter_context(nc.sbuf_tensor([P, 1], F32))
        q_sb = _es.enter_context(nc.sbuf_tensor([CQ, N], F32))
        k_sb = _es.enter_context(nc.sbuf_tensor([CQ, N], F32))
        vT_sb = _es.enter_context(nc.sbuf_tensor([P, NJ, C], F32))
        a2_sb = _es.enter_context(nc.sbuf_tensor([P, 4, NT], F32))
        acc_sb = _es.enter_context(nc.sbuf_tensor([P, 2, NT], F32))
        r_sb = _es.enter_context(nc.sbuf_tensor([1, 2, NT], F32))
        rb_sb = _es.enter_context(nc.sbuf_tensor([P, NT], F32))
        t1_sb = _es.enter_context(nc.sbuf_tensor([P, 2, NT], F32))
        ot0_sb = _es.enter_context(nc.sbuf_tensor([P, 2, NT], F32))
        ot1_sb = _es.enter_context(nc.sbuf_tensor([P, 2, NT], F32))
        pp0 = _es.enter_context(nc.psum_tensor([P, NT], F32))
        pp1 = _es.enter_context(nc.psum_tensor([P, NT], F32))
        s2a = _es.enter_context(nc.psum_tensor([P, NT], F32))
        s2b = _es.enter_context(nc.psum_tensor([P, NT], F32))
        oc0p = _es.enter_context(nc.psum_tensor([P, NT], F32))
        oc1p = _es.enter_context(nc.psum_tensor([P, NT], F32))
        srow = _es.enter_context(nc.psum_tensor([1, NT], F32))
        rbp = _es.enter_context(nc.psum_tensor([P, NT], F32))
        dsem = _es.enter_context(nc.semaphore())
        tsem = _es.enter_context(nc.semaphore())
        asem = _es.enter_context(nc.semaphore())
        vsem = _es.enter_context(nc.semaphore())
        block = _es.enter_context(nc.Block())
        pp = [pp0, pp1]
        s2p = [s2a, s2b]
        ocp = [oc0p, oc1p]

        @block.sync
        def _(sync):
            for dst, src in ((fa_sb[:], fa3), (fb_sb[:], fb3), (wq_sb[:], wq3),
                             (wk_sb[:], wk3), (wv_sb[:], wv3), (bq_sb[:], bqd[:]),
                             (bk_sb[:], bkd[:]), (bv_sb[:], bvd[:]),
                             (gam_sb[:], gamd[:])):
                sync.dma_start(dst, src).then_inc(dsem, 16)
            for it in range(NIT):
                isl = slice(it * NT, (it + 1) * NT)
                for cc, ot in ((0, ot0_sb), (1, ot1_sb)):
                    sync.wait_ge(asem, A0(it) + 34 + cc)
                    sync.dma_start(out3[:, cc, isl], ot[:, it % 2]).then_inc(dsem, 16)

        @block.tensor
        def _(tensor):
            tensor.wait_ge(dsem, DS0)
            tensor.wait_ge(vsem, VS0)
            # q, k tiles (n = 2t -> q, 2t+1 -> k)
            for n in range(16):
                t = n // 2
                sl = slice(t * NT, (t + 1) * NT)
                w = wq_sb if n % 2 == 0 else wk_sb
                if n >= 2:
                    tensor.wait_ge(asem, n - 1)
                pq = pp[n % 2][0:CQ]
                nc.tensor.matmul(pq, lhsT=w[:, 0], rhs=fb_sb[:, 0, sl],
                                 start=True, stop=False).then_inc(tsem, 1)
                nc.tensor.matmul(pq, lhsT=w[:, 1], rhs=fb_sb[:, 1, sl],
                                 start=False, stop=True).then_inc(tsem, 1)
            # vT tiles
            for n in range(NJ):
                jsl = slice(n * P, (n + 1) * P)
                tensor.wait_ge(asem, 16 + max(0, n - 1))
                pv = pp[n % 2][:, 0:C]
                nc.tensor.matmul(pv, lhsT=fa_sb[:, 0, jsl], rhs=wv_sb[:, 0],
                                 start=True, stop=False).then_inc(tsem, 1)
                nc.tensor.matmul(pv, lhsT=fa_sb[:, 1, jsl], rhs=wv_sb[:, 1],
                                 start=False, stop=False).then_inc(tsem, 1)
                nc.tensor.matmul(pv, lhsT=onesr[:], rhs=bv_sb[:],
                                 start=False, stop=True).then_inc(tsem, 1)
            # main loop
            for it in range(NIT):
                isl = slice(it * NT, (it + 1) * NT)

                def s2_mm(jj, it=it, isl=isl):
                    if jj < 2:
                        tensor.wait_ge(asem, AQKV if it == 0 else A0(it) - 3)
                    else:
                        tensor.wait_ge(asem, A0(it) + jj - 1)
                    jsl = slice(jj * P, (jj + 1) * P)
                    nc.tensor.matmul(s2p[jj % 2][:], lhsT=k_sb[:, jsl],
                                     rhs=q_sb[:, isl],
                                     start=True, stop=True).then_inc(tsem, 1)

                s2_mm(0)
                s2_mm(1)
                for jb in range(NJ):
                    if jb + 2 < NJ:
                        s2_mm(jb + 2)
                    tensor.wait_ge(asem, A0(it) + jb + 1)
                    if jb == 0 and it > 0:
                        tensor.wait_ge(vsem, V0(it))
                    nc.tensor.matmul(ocp[0][:], lhsT=vT_sb[:, jb, 0:P],
                                     rhs=a2_sb[:, jb % 4],
                                     start=(jb == 0), stop=(jb == NJ - 1)
                                     ).then_inc(tsem, 1)
                    nc.tensor.matmul(ocp[1][:], lhsT=vT_sb[:, jb, P:C],
                                     rhs=a2_sb[:, jb % 4],
                                     start=(jb == 0), stop=(jb == NJ - 1)
                                     ).then_inc(tsem, 1)
                tensor.wait_ge(vsem, V0(it) + 32)
                nc.tensor.matmul(srow[:], lhsT=onesc[:], rhs=acc_sb[:, it % 2],
                                 start=True, stop=True).then_inc(tsem, 1)
                tensor.wait_ge(vsem, V0(it) + 34)
                nc.tensor.matmul(rbp[:], lhsT=onesr[:], rhs=r_sb[:, it % 2],
                                 start=True, stop=True).then_inc(tsem, 1)

        @block.scalar
        def _(scalar):
            # q/k bias-add moves
            for n in range(16):
                t = n // 2
                sl = slice(t * NT, (t + 1) * NT)
                scalar.wait_ge(tsem, 2 * (n + 1))
                dst = q_sb if n % 2 == 0 else k_sb
                bias = bq_sb if n % 2 == 0 else bk_sb
                nc.scalar.activation(dst[:, sl], pp[n % 2][0:CQ], AF.Identity,
                                     bias=bias[:]).then_inc(asem, 1)
            # vT copies
            for n in range(NJ):
                scalar.wait_ge(tsem, 32 + 3 * (n + 1))
                nc.scalar.copy(vT_sb[:, n], pp[n % 2][:, 0:C]).then_inc(asem, 1)
            # main loop
            for it in range(NIT):
                for jb in range(NJ):
                    scalar.wait_ge(tsem, T0(it) + _pos_s2(jb))
                    if jb >= 4:
                        scalar.wait_ge(tsem, T0(it) + _pos_oc1(jb - 4))
                        scalar.wait_ge(vsem, V0(it) + jb - 3)
                    elif it > 0:
                        scalar.wait_ge(tsem, T0(it - 1) + _pos_oc1(jb + 28))
                        scalar.wait_ge(vsem, V0(it - 1) + jb + 29)
                    nc.scalar.activation(a2_sb[:, jb % 4], s2p[jb % 2][:], AF.Exp,
                                         bias=expb[:]).then_inc(asem, 1)
                scalar.wait_ge(tsem, T0(it) + 98)
                if it > 0:
                    scalar.wait_ge(vsem, V0(it))
                nc.scalar.copy(rb_sb[:], rbp[:]).then_inc(asem, 1)
                for cc, ot in ((0, ot0_sb), (1, ot1_sb)):
                    scalar.wait_ge(vsem, V0(it) + 36 + 2 * cc)
                    if it >= 2:
                        scalar.wait_ge(dsem, DS0 + 16 * 2 * (it - 1))
                    nc.scalar.activation(ot[:, it % 2], t1_sb[:, cc], AF.Relu
                                         ).then_inc(asem, 1)

        @block.vector
        def _(vector):
            nc.vector.memset(onesc[:], 1.0).then_inc(vsem, 1)
            nc.vector.memset(onesr[:], 1.0).then_inc(vsem, 1)
            nc.vector.memset(expb[:], EXP_BIAS).then_inc(vsem, 1)
            vector.wait_ge(dsem, DS0)
            for it in range(NIT):
                isl = slice(it * NT, (it + 1) * NT)
                for jb in range(NJ):
                    vector.wait_ge(asem, A0(it) + jb + 1)
                    if jb == 0:
                        if it >= 2:
                            vector.wait_ge(tsem, T0(it - 2) + 97)
                        nc.vector.tensor_copy(out=acc_sb[:, it % 2],
                                              in_=a2_sb[:, jb % 4]
                                              ).then_inc(vsem, 1)
                    else:
                        nc.vector.tensor_add(out=acc_sb[:, it % 2],
                                             in0=acc_sb[:, it % 2],
                                             in1=a2_sb[:, jb % 4]
                                             ).then_inc(vsem, 1)
                vector.wait_ge(tsem, T0(it) + 97)
                nc.vector.reciprocal(r_sb[:, it % 2], srow[:]).then_inc(vsem, 1)
                nc.vector.tensor_scalar_mul(r_sb[:, it % 2], r_sb[:, it % 2],
                                            gam_sb[0:1]).then_inc(vsem, 1)
                vector.wait_ge(tsem, T0(it) + 96)
                vector.wait_ge(asem, A0(it) + 33)
                for cc in (0, 1):
                    nc.vector.tensor_mul(out=t1_sb[:, cc], in0=ocp[cc][:],
                                         in1=rb_sb[:]).then_inc(vsem, 1)
                    nc.vector.tensor_add(out=t1_sb[:, cc], in0=t1_sb[:, cc],
                                         in1=fa_sb[:, cc, isl]).then_inc(vsem, 1)

    return nc


def _get_nc():
    if "nc" not in _CACHE:
        _CACHE["nc"] = _build()
    return _CACHE["nc"]


def kernel(**inputs):
    fa = np.asarray(inputs["fa"], dtype=np.float32)
    fb = np.asarray(inputs["fb"], dtype=np.float32)
    Wq = np.asarray(inputs["Wq"], dtype=np.float32)
    Wk = np.asarray(inputs["Wk"], dtype=np.float32)
    Wv = np.asarray(inputs["Wv"], dtype=np.float32)
    bq = np.asarray(inputs["bq"], dtype=np.float32)
    bk = np.asarray(inputs["bk"], dtype=np.float32)
    bv = np.asarray(inputs["bv"], dtype=np.float32)
    gamma = float(np.asarray(inputs["gamma"]))

    wqT = np.ascontiguousarray(Wq.T)
    wkT = np.ascontiguousarray(Wk.T)
    wvT = np.ascontiguousarray(Wv.T)
    bq2 = np.ascontiguousarray(bq.reshape(CQ, 1))
    bk2 = np.ascontiguousarray(bk.reshape(CQ, 1))
    bv2 = np.ascontiguousarray(bv.reshape(1, C))
    gam2 = np.full((P, 1), gamma, dtype=np.float32)

    in_maps = []
    for b in range(B):
        in_maps.append({
            "fa": np.ascontiguousarray(fa[b].reshape(C, N)),
            "fb": np.ascontiguousarray(fb[b].reshape(C, N)),
            "wqT": wqT, "wkT": wkT, "wvT": wvT,
            "bq": bq2, "bk": bk2, "bv": bv2, "gamma": gam2,
        })

    nc = _get_nc()
    _CACHE["in_maps"] = in_maps
    res = run_bass_kernel_spmd(nc, in_maps, list(range(B))).results
    out = np.stack([res[b]["out"].reshape(C, HW, HW) for b in range(B)])
    return out.astype(np.float32)

